# revision 18
# baseline (speedup 1.0000x reference)
"""Self-contained Trainium2 Bass kernel for nn_AttLayer_model_5.

kernel(**inputs) takes the FULL unsharded inputs (B=64, T=2048, D=256, H=5),
shards the batch across 8 NeuronCores (data-parallel, 8 samples/core),
runs a Bass/Tile kernel via concourse.bass_utils.run_bass_kernel_spmd,
and gathers the full (64, 256) float32 output.

Math (per sample):
  temp  = x @ W_temp + b_temp          # (T,H), contraction over D
  fea   = xfea[:,None]*W_fea[0] + b_fea
  had   = tanh(temp) * tanh(fea)
  inter = had @ v, v = uw.sum(1)       # sum(b) shift dropped: softmax-invariant
  e     = exp(inter)                   # no max-subtraction: |inter| <~ 0.03
  wnum  = e * mask
  y     = (wnum @ x) / sum(wnum)       # (D,)

Device strategy (per core, 8 samples). Per-iteration HBM traffic is the
roofline: x fp16 8 MiB (pooling operand, token-partition layout t=16p+c)
+ xT fp8 4 MiB (projection operand, D-partition layout; fp8 is safe on
the projection path only: d(y)/d(inter) ~ 0.01 so fp8's 3.6% rms on temp
lands ~2e-5 in y; measured: fp8 on the POOLING operand costs 2.65e-2 —
over the gate — so that copy stays fp16) + xfea/mask ~48 KiB. All
weight-derived constants are hoisted OUT of the benchmark loop (loaded
once). Both host repacks are fully partition-contiguous; loads stream as
2 MiB pieces alternating between the two HWDGE rings (SP + ACT) — the
SWDGE/gpsimd path measured 12-15% slower, and this split measured
fastest of {1-ring, 2-ring-by-tensor, 6/6-balanced, SWDGE-mix}.

Compute per iteration (PE exec ~30us at 88% occupancy < DMA ~37us):
- projection packs 4 samples per PSUM tile at partition offsets 32*j via
  matmul tile_position; both groups' V-matmuls accumulate one (8, 512)
  inter tile; biases ride ACT activations as per-partition bias patterns;
  mask adds bf16; exp banks per-stripe denominators via accum_out.
- pooling trails one stripe: e 8-col PE transposes -> fp16 wts -> 1-row
  matmuls accumulating fp32 in PSUM over 16 token chunks; 1/sum(wnum)
  lands in two full-width scaled copies gathered by a single
  partition-strided y DMA.

_get_module(n_iters) with n_iters > 1 builds a 2-stage software-pipelined
loop (tc.For_i_pipelined, unroll=2): iteration k+1's input DMAs stream
underneath iteration k's compute, so the sustained per-iteration time is
max(DMA, compute) rather than their partial sum. Input tiles are
double-buffered except x chunks 0-7 (single-buffered for SBUF; their
reload is WAR-gated on the previous iteration's early pool stripes and
ordered last on both rings so the gate is already open). The test
harness measures sustained per-execution device time as the marginal
cost of extra iterations — host dispatch and axon tunnel latency cancel.

Measured on HW (8 cores): rel err 4.7e-4; 37.5us/exec (For_i marginal)
vs 37.0us for the load stream alone and ~35.3us device-HBM roofline
(8 cores x 12.05 MiB / 2.86 TB/s) — ~95% of device HBM peak. Session
history: 51.9us baseline -> 41.3 (pipelined loop) -> 38.5 (HWDGE-only
loads, contiguous repacks, consts hoisted) -> 37.5 (fine ring split).
"""

import os
import sys
from contextlib import ExitStack

import numpy as np

for _p in ("/opt/trn_rl_repo", "/root/.axon_site/_ro/trn_rl_repo"):
    if os.path.isdir(_p) and _p not in sys.path:
        sys.path.insert(0, _p)
        break

import ml_dtypes

import concourse.bass as bass
import concourse.mybir as mybir
import concourse.tile as tile
from concourse import bacc
from concourse.bass_utils import run_bass_kernel_spmd

F32 = mybir.dt.float32
F16 = mybir.dt.float16
BF16 = mybir.dt.bfloat16
F8 = mybir.dt.float8e4

NP_BF16 = ml_dtypes.bfloat16
NP_F8 = ml_dtypes.float8_e4m3

N_CORES = 8
B = 64
B_LOC = B // N_CORES  # 8 samples per core
T = 2048
D = 256
H = 5
NC16 = T // 128  # 16 token chunks per sample
NQ = T // 512    # 4 stripes
AF = mybir.ActivationFunctionType
ALU = mybir.AluOpType

# bump on any kernel change: pad's shape keys the HLO hash, defeating a
# stale compile-cache NEFF for an unchanged-io, changed-body program
KERNEL_VERSION = 42


def _host_constants(W_temp, b_temp, W_fea, b_fea, uw):
    """Pure O(D*H + H^2) weight repacking on host, pre-cast to compute dtypes."""
    W_temp = np.asarray(W_temp, np.float32)
    b_temp = np.asarray(b_temp, np.float32)
    W_fea = np.asarray(W_fea, np.float32)
    b_fea = np.asarray(b_fea, np.float32)
    uw = np.asarray(uw, np.float32)

    v = uw.sum(axis=1)

    wt = np.zeros((128, 64), np.float32)
    wt[:, 0:H] = W_temp[:128]
    wt[:, 32 : 32 + H] = W_temp[128:]

    vpat = np.zeros((128, 16), np.float32)
    for s in range(B_LOC):
        g, j = divmod(s, 4)
        vpat[32 * j : 32 * j + H, 8 * g + s] = v

    fpat = np.zeros((4, 128), np.float32)
    for j in range(4):
        fpat[j, 32 * j : 32 * j + H] = W_fea[0]

    btpat = np.zeros((128, 1), np.float32)
    bfpat = np.zeros((128, 1), np.float32)
    for j in range(4):
        btpat[32 * j : 32 * j + H, 0] = b_temp
        bfpat[32 * j : 32 * j + H, 0] = b_fea

    patg = np.zeros((8, 256), np.float32)
    for g in range(2):
        for j in range(4):
            patg[4 * g + j, 128 * g + 32 * j] = 1.0

    # pack per dtype into one blob each:
    # c8:  wt [128, 0:64] | fpat [0:4, 64:192]
    # cb16: vpat [128, 0:16]
    # c32: btpat [128, 0:1] | bfpat [128, 1:2] | ident8 [0:8, 2:10]
    #      | patg [0:8, 10:266]
    c8s = np.zeros((128, 192), NP_F8)
    c8s[:, 0:64] = wt.astype(NP_F8)
    c8s[0:4, 64:192] = fpat.astype(NP_F8)
    cb16 = np.zeros((128, 16), NP_BF16)
    cb16[:, 0:16] = vpat.astype(NP_BF16)
    c32 = np.zeros((128, 266), np.float32)
    c32[:, 0:1] = btpat
    c32[:, 1:2] = bfpat
    c32[0:8, 2:10] = np.eye(8, dtype=np.float32)
    c32[0:8, 10:266] = patg
    return {"c8s": c8s, "cb16": cb16, "c32": c32}


def _declare_io(nc, n_iters):
    io = {}
    # x fp16, pooling operand: [p, c, s, d], token t = 16p + c.
    # Fully partition-contiguous (64 KiB/partition).
    io["x"] = nc.dram_tensor("x", [128, NC16, B_LOC, D], F16, kind="ExternalInput")
    # xT fp8, projection operand: [d(partition), q, dh, s, 128i+p],
    # token t = 16p + 4q + i. Fully partition-contiguous (32 KiB each),
    # loads as ONE 4 MiB DMA.
    io["xt"] = nc.dram_tensor(
        "xt", [128, NQ, 2, B_LOC, 512], F8, kind="ExternalInput"
    )
    io["c8s"] = nc.dram_tensor("c8s", [128, 192], F8, kind="ExternalInput")
    io["xfea8"] = nc.dram_tensor(
        "xfea8", [4, 2 * T], F8, kind="ExternalInput"
    )
    io["cb16"] = nc.dram_tensor("cb16", [128, 16], BF16, kind="ExternalInput")
    io["masku"] = nc.dram_tensor(
        "masku", [B_LOC, T], BF16, kind="ExternalInput"
    )
    io["c32"] = nc.dram_tensor("c32", [128, 266], F32, kind="ExternalInput")
    # never read: its shape keys the HLO hash (see KERNEL_VERSION)
    io["pad"] = nc.dram_tensor(
        "pad", [1, KERNEL_VERSION * 257 + n_iters], F32, kind="ExternalInput"
    )
    io["y"] = nc.dram_tensor("y", [B_LOC, D], F32, kind="ExternalOutput")
    return io


def _emit_consts(nc, tc, ctx, io):
    """Weight-derived constants: loaded once, outside the benchmark loop."""
    cpool = ctx.enter_context(tc.tile_pool(name="consts", bufs=1))
    c8s_sb = cpool.tile([128, 192], F8, name="c8s_sb")
    nc.scalar.dma_start(c8s_sb[:], io["c8s"].ap()[:])
    cb16_sb = cpool.tile([128, 16], BF16, name="cb16_sb")
    nc.scalar.dma_start(cb16_sb[:], io["cb16"].ap()[:])
    c32_sb = cpool.tile([128, 266], F32, name="c32_sb")
    nc.scalar.dma_start(c32_sb[:], io["c32"].ap()[:])
    return {
        "wt": c8s_sb[:, 0:64],
        "fpat": c8s_sb[0:4, 64:192],
        "vpat": cb16_sb[:, 0:16],
        "ident8": c32_sb[0:8, 2:10],
        "btpat": c32_sb[:, 0:1],
        "bfpat": c32_sb[:, 1:2],
        "patg": c32_sb[0:8, 10:266],
    }


def _alloc_work(nc, tc, ctx):
    """Per-iteration working tiles + PSUM pools, allocated once."""
    w = {}
    e_pool = ctx.enter_context(tc.tile_pool(name="epool", bufs=1))
    w["e_sb"] = e_pool.tile([B_LOC, T], F32, name="e_sb")
    w["den4_sb"] = e_pool.tile([B_LOC, NQ], F32, name="den4_sb")
    w["den_sb"] = e_pool.tile([B_LOC, 1], F32, name="den_sb")
    w["rec_sb"] = e_pool.tile([B_LOC, 1], F32, name="rec_sb")

    w["ttp_pool"] = ctx.enter_context(
        tc.tile_pool(name="ttp", bufs=2, space="PSUM")
    )
    w["fep_pool"] = ctx.enter_context(
        tc.tile_pool(name="fep", bufs=1, space="PSUM")
    )
    w["itp_pool"] = ctx.enter_context(
        tc.tile_pool(name="itp", bufs=2, space="PSUM")
    )
    w["act_pool"] = ctx.enter_context(tc.tile_pool(name="acts", bufs=2))
    # phase-3 accumulators: wtp and ypp0 share one bank-tile, ypp1 its own
    p3_pool = ctx.enter_context(tc.tile_pool(name="p3", bufs=1, space="PSUM"))
    combo = p3_pool.tile([128, 512], F32, name="combo")
    w["wtp"] = combo[:, 0:128]
    w["ypps"] = [combo[:, 128:384], p3_pool.tile([128, D], F32, name="ypp1")]
    w["recp"] = combo[:, 384:386]
    out_pool = ctx.enter_context(tc.tile_pool(name="outp", bufs=1))
    w["wts"] = out_pool.tile([128, 128], F16, name="wts")
    w["recs"] = out_pool.tile([128, 2], F32, name="recs")
    w["y_scat"] = out_pool.tile([128, 2 * D], F32, name="y_scat")
    return w


# all input DMAs ride HWDGE rings (SWDGE's Q7 descriptor path measured
# ~12-15% slower at this transfer mix). LOADQ="fine" (default, fastest
# measured) streams the 12 MiB as 2 MiB pieces alternating between the
# SP and ACT rings; "2h" puts xt+small on ACT and x on SP; "2hb"
# balances the rings 6/6 MiB; "sp1" puts everything on SP.
LOADQ = os.environ.get("LOADQ", "fine")
UNROLL = int(os.environ.get("UNROLL", "2"))


# input-tile order: xt, x23, x01, xfea, masku
def _alloc_inputs(alloc):
    xt = alloc([128, NQ * 2 * B_LOC * 512], F8, "xt", None)
    x23 = alloc([128, 8 * B_LOC * D], F16, "x23", None)
    # chunks 0-7 single-buffered: reload is WAR-gated on the previous
    # iteration's pool stripes 0/1, which retire early; ordered last on SP
    x01 = alloc([128, 8 * B_LOC * D], F16, "x01", 1)
    xfea = alloc([4, 2 * T], F8, "xfea", None)
    masku = alloc([B_LOC, T], BF16, "masku", None)
    return (xt, x23, x01, xfea, masku)


def _emit_loads(nc, io, tiles):
    xt, x23, x01, xfea, masku = tiles
    xsrc = io["x"].ap()
    xtf = io["xt"].ap().rearrange("p q dh s t -> p (q dh s t)")
    half = 4 * B_LOC * D
    nq = NQ * 2 * B_LOC * 512
    if LOADQ == "fine":
        # 2 MiB pieces alternating between the two HWDGE rings; the
        # WAR-gated x01 halves (pool stripes 0/1 of the previous tick)
        # tail both rings, when their gates are already open.
        nc.scalar.dma_start(xfea[:], io["xfea8"].ap()[:])
        nc.scalar.dma_start(masku[:], io["masku"].ap()[:])
        nc.scalar.dma_start(xt[:, 0 : nq // 2], xtf[:, 0 : nq // 2])
        nc.sync.dma_start(xt[:, nq // 2 : nq], xtf[:, nq // 2 : nq])
        nc.sync.dma_start(
            x23[:, 0:half], xsrc[:, 8:12].rearrange("p c s d -> p (c s d)")
        )
        nc.scalar.dma_start(
            x23[:, half : 2 * half],
            xsrc[:, 12:16].rearrange("p c s d -> p (c s d)"),
        )
        nc.sync.dma_start(
            x01[:, 0:half], xsrc[:, 0:4].rearrange("p c s d -> p (c s d)")
        )
        nc.scalar.dma_start(
            x01[:, half : 2 * half],
            xsrc[:, 4:8].rearrange("p c s d -> p (c s d)"),
        )
        return
    eng_a = nc.sync if LOADQ == "sp1" else nc.scalar
    # small per-iteration tensors first (needed at the very start of the
    # next compute tick), then the three big streams
    eng_a.dma_start(xfea[:], io["xfea8"].ap()[:])
    eng_a.dma_start(masku[:], io["masku"].ap()[:])
    eng_a.dma_start(xt[:], xtf)
    # x chunks 8-15 (double-buffered, no WAR gate) first, then chunks 0-7
    # whose single-buffer WAR gate (pool stripes 0/1 of the previous
    # tick) is open by the time the queues reach them.
    nc.sync.dma_start(
        x23[:], xsrc[:, 8:16].rearrange("p c s d -> p (c s d)")
    )
    if LOADQ == "2hb":
        # balance the rings 6/6: chunks 0-3 tail the SP ring, 4-7 tail ACT
        nc.sync.dma_start(
            x01[:, 0:half], xsrc[:, 0:4].rearrange("p c s d -> p (c s d)")
        )
        nc.scalar.dma_start(
            x01[:, half : 2 * half],
            xsrc[:, 4:8].rearrange("p c s d -> p (c s d)"),
        )
    else:
        nc.sync.dma_start(
            x01[:], xsrc[:, 0:8].rearrange("p c s d -> p (c s d)")
        )


def _emit_compute(nc, tc, io, cn, w, tiles):
    mm = nc.tensor.matmul
    xt, x23, x01, xfea, masku = tiles

    xt_av = xt[:].rearrange("p (q dh s t) -> p q dh s t", q=NQ, dh=2, s=B_LOC)
    xt_v = [xt_av[:, q] for q in range(NQ)]
    x01_v = x01[:].rearrange("p (c s d) -> p c s d", c=8, s=B_LOC)
    x23_v = x23[:].rearrange("p (c s d) -> p c s d", c=8, s=B_LOC)

    def x_chunk(c, s):
        if c < 8:
            return x01_v[:, c, s, :]
        return x23_v[:, c - 8, s, :]

    e_sb = w["e_sb"]
    den4_sb = w["den4_sb"]
    act_pool = w["act_pool"]
    wtp = w["wtp"]
    ypps = w["ypps"]
    wts = w["wts"]

    # zero the pooling accumulators' unwritten partitions once per
    # iteration (on DVE, idle early) so the full-width y gather reads
    # defined values
    for g in range(2):
        nc.vector.memset(ypps[g][:, :], 0.0)

    # tanh(fea) for each (stripe, group), emitted as fillers inside the
    # projection so ACT works while PE streams matmuls
    tfs_all = {}

    def emit_tfs(q):
        """both groups' tanh(fea) for stripe q in one wide PSUM tile and
        one ACT op"""
        fep = w["fep_pool"].tile([128, 2 * 512], F32, name=f"fep{q}", tag="fep")
        for g in range(2):
            mm(
                fep[:, bass.ds(g * 512, 512)],
                cn["fpat"],
                xfea[0:4, bass.ds(g * T + 512 * q, 512)],
                skip_group_check=True,
            )
        tfs = act_pool.tile(
            [128, 2 * 512], BF16, name=f"tfs{q}", tag="tfs", bufs=4
        )
        nc.scalar.activation(tfs[:], fep[:], AF.Tanh, bias=cn["bfpat"])
        for g in range(2):
            tfs_all[(q, g)] = tfs[:, bass.ds(g * 512, 512)]

    tfs_todo = list(range(NQ))

    def proj_group(q, g):
        """packed projection MMs from the shipped fp8 xT stripe tiles."""
        ttp = w["ttp_pool"].tile([128, 512], F32, name=f"ttp{q}{g}", tag="ttp")
        for dh in range(2):
            for j in range(4):
                s = 4 * g + j
                mm(
                    ttp[32 * j : 32 * j + 32, :],
                    cn["wt"][:, 32 * dh : 32 * dh + 32],
                    xt_v[q][:, dh, s, :],
                    start=(dh == 0),
                    stop=(dh == 1),
                    tile_position=(0, 32 * j),
                    skip_group_check=True,
                )
        if g == 0 and tfs_todo:
            emit_tfs(tfs_todo.pop(0))
        return ttp

    def tanh_had_v(q, g, ttp, itp):
        """tanh(temp), hadamard with precomputed tanh(fea), V-matmul
        accumulating both groups into one (8, 512) PSUM tile."""
        tts = act_pool.tile([128, 512], BF16, name=f"tts{q}{g}", tag="tts")
        nc.scalar.activation(tts[:], ttp[:], AF.Tanh, bias=cn["btpat"])
        had = act_pool.tile([128, 512], BF16, name=f"had{q}{g}", tag="had")
        nc.vector.tensor_mul(had[:], tts[:], tfs_all[(q, g)])
        mm(
            itp[:8, :],
            cn["vpat"][:, 8 * g : 8 * g + 8],
            had[:],
            start=(g == 0),
            stop=(g == 1),
            skip_group_check=True,
        )

    def pool_stripe(q):
        """w-transposes + packed fp16 pooling MMs for stripe q."""
        for i in range(4):
            c = 4 * q + i
            mm(
                wtp[:, 8 * c : 8 * c + 8],
                e_sb[:, 128 * c : 128 * (c + 1)],
                cn["ident8"],
                is_transpose=True,
                start=(c == 0),
                stop=(c == NC16 - 1),
                skip_group_check=True,
            )
        nc.vector.tensor_copy(
            wts[:, 32 * q : 32 * (q + 1)], wtp[:, 32 * q : 32 * (q + 1)]
        )
        for i in range(4):
            c = 4 * q + i
            for g in range(2):
                for j in range(4):
                    s = 4 * g + j
                    mm(
                        ypps[g][32 * j : 32 * j + 1, :],
                        wts[:, 8 * c + s : 8 * c + s + 1],
                        x_chunk(c, s),
                        start=(c == 0),
                        stop=(c == NC16 - 1),
                        tile_position=(0, 32 * j),
                        skip_group_check=True,
                    )

    # ---- single-phase pipeline: per stripe, both groups' projections,
    # tanh/hadamard, V-accumulation, mask+exp; pooling trails one stripe
    # so its matmuls fill the next stripe's cross-engine stalls ----
    for q in range(NQ):
        itp = w["itp_pool"].tile([128, 512], F32, name=f"itp{q}", tag="itp")
        ttps = [proj_group(q, 0), proj_group(q, 1)]
        for g in range(2):
            tanh_had_v(q, g, ttps[g], itp)
        if q >= 1:
            pool_stripe(q - 1)
        inter = act_pool.tile([8, 512], F32, name=f"inter{q}", tag="inter")
        nc.vector.tensor_add(
            inter[:], itp[:8, :], masku[0:B_LOC, bass.ds(512 * q, 512)]
        )
        nc.scalar.activation(
            e_sb[:, bass.ds(512 * q, 512)],
            inter[:],
            AF.Exp,
            accum_out=den4_sb[:, q : q + 1],
        )
    pool_stripe(NQ - 1)

    # ---- finale: denominators -> reciprocal patterns -> scaled gather
    # (the recp matmul shares combo's PSUM bank with the pooling
    # accumulators, so it must not run before the last pool stripe) ----
    nc.vector.tensor_reduce(
        w["den_sb"][:], den4_sb[:], axis=mybir.AxisListType.X, op=ALU.add
    )
    nc.vector.reciprocal(w["rec_sb"][:], w["den_sb"][:])
    for g in range(2):
        mm(
            w["recp"][:, g : g + 1],
            cn["patg"][:, 128 * g : 128 * (g + 1)],
            w["rec_sb"][:],
        )
    recs = w["recs"]
    nc.vector.tensor_copy(recs[:], w["recp"][:])

    # one full-width scaled copy per group (sample rows live at partitions
    # 32j; other partitions carry the zeros memset at body start and are
    # never shipped), then a single partition-strided DMA gathers the
    # 4 sample rows of both group column-blocks
    y_scat = w["y_scat"]
    for g in range(2):
        nc.scalar.mul(
            y_scat[:, bass.ds(g * D, D)], ypps[g][:, :], recs[:, g : g + 1]
        )
    src = (
        y_scat[:]
        .rearrange("(j r) (g d) -> j r g d", r=32, g=2)[:, 0, :, :]
    )
    nc.scalar.dma_start(
        io["y"].ap().rearrange("(g j) d -> j g d", g=2), src
    )


def _build(nc, tc, io, ctx, n_iters):
    cn = _emit_consts(nc, tc, ctx, io)
    w = _alloc_work(nc, tc, ctx)
    if n_iters == 1:
        pool = ctx.enter_context(tc.tile_pool(name="inp", bufs=1))

        def alloc(shape, dt, name, bufs):
            return pool.tile(shape, dt, name=name)

        tiles = _alloc_inputs(alloc)
        _emit_loads(nc, io, tiles)
        _emit_compute(nc, tc, io, cn, w, tiles)
    else:

        def load(pipe, iv):
            def alloc(shape, dt, name, bufs):
                kw = {} if bufs is None else {"bufs": bufs}
                return pipe.intermediate_tile(shape, dt, name=name, **kw)

            tiles = _alloc_inputs(alloc)
            _emit_loads(nc, io, tiles)
            return tiles

        def compute(pipe, iv, tiles):
            _emit_compute(nc, tc, io, cn, w, tiles)

        tc.For_i_pipelined(
            [load, compute], 0, n_iters, unroll=UNROLL, staged_num_bufs=2
        )


_MODULE_CACHE = {}


def _get_module(n_iters=1):
    if n_iters not in _MODULE_CACHE:
        nc = bacc.Bacc("TRN2", target_bir_lowering=False, debug=False)
        io = _declare_io(nc, n_iters)
        with tile.TileContext(nc) as tc:
            with ExitStack() as ctx:
                _build(nc, tc, io, ctx, n_iters)
        nc.compile()
        _MODULE_CACHE[n_iters] = nc
    return _MODULE_CACHE[n_iters]


def make_in_maps(
    x_temp, x_fea, mask, W_temp, b_temp, W_fea, b_fea, b, uw, n_iters=1
):
    """Shard full inputs into per-core input maps (host-side, O(bytes))."""
    x_temp = np.ascontiguousarray(np.asarray(x_temp, np.float32))
    x_fea = np.asarray(x_fea, np.float32)
    masku = np.asarray(mask).astype(np.uint8)
    consts = _host_constants(W_temp, b_temp, W_fea, b_fea, uw)

    # x fp16 pooling copy: [core, p, c, s, d], token t = 16p + c
    x16 = np.ascontiguousarray(
        x_temp.reshape(N_CORES, B_LOC, 128, NC16, D).transpose(0, 2, 3, 1, 4)
    ).astype(np.float16)
    # xT fp8 projection copy: [core, d, q, dh, s, 128i + p], t = 16p+4q+i
    xt8 = np.ascontiguousarray(
        x_temp.reshape(N_CORES, B_LOC, 128, 4, 4, 2, 128)
        .transpose(0, 6, 3, 5, 1, 4, 2)
        .reshape(N_CORES, 128, NQ, 2, B_LOC, 512)
    ).astype(NP_F8)

    in_maps = []
    for k in range(N_CORES):
        sl = slice(k * B_LOC, (k + 1) * B_LOC)
        xfea_p = (
            x_fea[sl].reshape(B_LOC, 128, NC16).swapaxes(1, 2).reshape(B_LOC, T)
        )
        xfea_k = (
            xfea_p
            .reshape(2, 4, T)
            .swapaxes(0, 1)
            .reshape(4, 2 * T)
        )
        xfea8_k = np.ascontiguousarray(xfea_k).astype(NP_F8)
        masku_k = np.where(
            masku[sl].reshape(B_LOC, 128, NC16)
            .swapaxes(1, 2)
            .reshape(B_LOC, T)
            != 0,
            np.float32(0.0),
            np.float32(-1e30),
        ).astype(NP_BF16)
        in_maps.append(
            {
                "pad": np.zeros(
                    (1, KERNEL_VERSION * 257 + n_iters), np.float32
                ),
                "x": x16[k],
                "xt": xt8[k],
                "c8s": consts["c8s"],
                "xfea8": xfea8_k,
                "cb16": consts["cb16"],
                "masku": masku_k,
                "c32": consts["c32"],
            }
        )
    return in_maps


def kernel(x_temp, x_fea, mask, W_temp, b_temp, W_fea, b_fea, b, uw):
    nc = _get_module()
    in_maps = make_in_maps(
        x_temp, x_fea, mask, W_temp, b_temp, W_fea, b_fea, b, uw
    )
    res = run_bass_kernel_spmd(nc, in_maps, list(range(N_CORES)))
    return np.concatenate([res.results[k]["y"] for k in range(N_CORES)], axis=0)
